# revision 7
# baseline (speedup 1.0000x reference)
"""Trainium2 Bass kernel for nn_Memory_27882927686265 (scatter_memory).

Per-class sort-merge queue update:
  concat 1024 queue scores + 512 input scores, stable-descending top-1024,
  gather the corresponding 512-wide mu rows, scatter back per class.

Sharding: 200 classes split 25-per-core across 8 NeuronCores; inp_mu
replicated per core.

Device algorithm per core (classes on partitions 0..24):
  1. Copy queue-mu rows + inp_mu into one Internal DRAM slab (indirect DMA
     under this runtime only resolves dynamic offsets against Internal
     tensors, not ExternalInputs), giving a single gather index space.
  2. DVE iterative top-8 (max / max_index / match_replace), 128 rounds ->
     stable descending sort of all 1536 scores per class (ties resolved by
     ascending index, matching jax.lax.top_k).
  3. Per 128-rank block: DVE 32x32 block-transpose of the index block to a
     partition-major [128, 25] layout, map local indices to slab rows, then
     per class one indirect DMA gathers 128 rows (2KB each) into SBUF and a
     contiguous DMA stores them to the output.
"""

import threading

import numpy as np

N_CLASS = 200
N_MU = 1024
D = 512
K = 512
N_CORES = 8
CPC = N_CLASS // N_CORES  # classes per core = 25
NTOT = N_MU + K  # 1536
N_SRC_ROWS = CPC * N_MU + K  # 26112
INP_BASE = CPC * N_MU  # 25600
N_BLOCKS = N_MU // 128  # 8

_lock = threading.Lock()
_cache = {}


def _build_nc():
    import concourse.bacc as bacc
    import concourse.mybir as mybir
    import concourse.tile as tile
    from concourse import bass

    nc = bacc.Bacc(
        "TRN2",
        target_bir_lowering=False,
        debug=False,
        num_devices=N_CORES,
    )

    qmu = nc.dram_tensor("qmu", [INP_BASE, D], mybir.dt.float32, kind="ExternalInput")
    impu = nc.dram_tensor("impu", [K, D], mybir.dt.float32, kind="ExternalInput")
    qsc = nc.dram_tensor("qsc", [CPC, N_MU], mybir.dt.float32, kind="ExternalInput")
    isc = nc.dram_tensor("isc", [CPC, K], mybir.dt.float32, kind="ExternalInput")
    out_mu = nc.dram_tensor(
        "out_mu", [CPC, N_MU, D], mybir.dt.float32, kind="ExternalOutput"
    )
    out_sc = nc.dram_tensor(
        "out_sc", [CPC, N_MU], mybir.dt.float32, kind="ExternalOutput"
    )
    # Internal slab: [queue rows of all 25 classes | inp_mu rows].
    islab = nc.dram_tensor("islab", [N_SRC_ROWS, D], mybir.dt.float32)

    with tile.TileContext(nc) as tc, tc.tile_pool(name="persist", bufs=1) as pp:
        # Fill the slab (DRAM->DRAM); overlaps with the selection loop.
        nc.sync.dma_start(islab.ap()[:INP_BASE, :], qmu.ap())
        nc.sync.dma_start(islab.ap()[INP_BASE:, :], impu.ap())

        # Persistent tiles.
        s_tile = pp.tile([CPC, NTOT], mybir.dt.float32, name="scores", tag="scores")
        sc_sorted = pp.tile(
            [CPC, N_MU], mybir.dt.float32, name="sc_sorted", tag="sc_sorted"
        )
        # Per-block index tiles: [32, 128] so the DVE 32x32 block transpose
        # applies directly; only rows :25 carry data.
        idx_blk = [
            pp.tile([32, 128], mybir.dt.uint32, name=f"idx_blk{b}", tag=f"idx_blk{b}")
            for b in range(N_BLOCKS)
        ]
        idx_blk_t = [
            pp.tile(
                [32, 128], mybir.dt.uint32, name=f"idx_blk_t{b}", tag=f"idx_blk_t{b}"
            )
            for b in range(N_BLOCKS)
        ]
        # Per-column class base (1024*c), as f32 for the DVE float ALU.
        base_cls = pp.tile([128, CPC], mybir.dt.float32, name="base_cls", tag="base")

        nc.gpsimd.iota(
            base_cls[:],
            pattern=[[N_MU, CPC]],
            base=0,
            channel_multiplier=0,
            allow_small_or_imprecise_dtypes=True,
        )
        for b in range(N_BLOCKS):
            nc.gpsimd.memset(idx_blk[b][:], 0)

        # Load scores: [q | inp] per class.
        nc.sync.dma_start(s_tile[:, :N_MU], qsc.ap())
        nc.sync.dma_start(s_tile[:, N_MU:], isc.ap())

        # Stable descending selection, 8 at a time.
        for t in range(N_MU // 8):
            b, w = divmod(t, 16)
            mx = sc_sorted[:CPC, 8 * t : 8 * t + 8]
            nc.vector.max(out=mx, in_=s_tile[:CPC, :])
            nc.vector.max_index(
                out=idx_blk[b][:CPC, 8 * w : 8 * w + 8],
                in_max=mx,
                in_values=s_tile[:CPC, :],
            )
            if t != N_MU // 8 - 1:
                nc.vector.match_replace(
                    out=s_tile[:CPC, :],
                    in_to_replace=mx,
                    in_values=s_tile[:CPC, :],
                    imm_value=-1.0,
                )

        with (
            tc.tile_pool(name="stage", bufs=8) as stage_pool,
            tc.tile_pool(name="idxg", bufs=2) as idx_pool,
        ):
            for b in range(N_BLOCKS):
                # Transpose [25,128] block (padded to 32 rows) to partition-major.
                nc.vector.transpose(out=idx_blk_t[b][:], in_=idx_blk[b][:])
                tpos = idx_pool.tile([128, CPC], mybir.dt.float32, tag="tpos")
                for g in range(4):
                    nc.vector.tensor_copy(
                        out=tpos[32 * g : 32 * g + 32, :],
                        in_=idx_blk_t[b][:, 32 * g : 32 * g + CPC],
                    )
                # Slab row: idx < 1024 -> 1024*c + idx ; else idx - 1024 + 25600
                mask = idx_pool.tile([128, CPC], mybir.dt.uint32, tag="mask")
                addq = idx_pool.tile([128, CPC], mybir.dt.float32, tag="addq")
                gidxf = idx_pool.tile([128, CPC], mybir.dt.float32, tag="gidxf")
                gidx = idx_pool.tile([128, CPC], mybir.dt.int32, tag="gidx")
                nc.vector.tensor_scalar(
                    mask[:], tpos[:], float(N_MU), None, op0=mybir.AluOpType.is_lt
                )
                nc.vector.tensor_tensor(
                    out=addq[:], in0=tpos[:], in1=base_cls[:], op=mybir.AluOpType.add
                )
                nc.vector.tensor_scalar(
                    gidxf[:],
                    tpos[:],
                    float(INP_BASE - N_MU),
                    None,
                    op0=mybir.AluOpType.add,
                )
                nc.vector.copy_predicated(gidxf[:], mask[:], addq[:])
                nc.vector.tensor_copy(out=gidx[:], in_=gidxf[:])

                for c in range(CPC):
                    stage = stage_pool.tile([128, D], mybir.dt.float32, tag="stage")
                    nc.gpsimd.indirect_dma_start(
                        out=stage[:],
                        out_offset=None,
                        in_=islab.ap(),
                        in_offset=bass.IndirectOffsetOnAxis(
                            ap=gidx[:, c : c + 1], axis=0
                        ),
                    )
                    nc.sync.dma_start(
                        out_mu.ap()[c, 128 * b : 128 * (b + 1), :], stage[:]
                    )

        nc.sync.dma_start(out_sc.ap(), sc_sorted[:CPC, :])

    nc.compile()
    return nc


def get_nc():
    with _lock:
        if "nc" not in _cache:
            _cache["nc"] = _build_nc()
        return _cache["nc"]


def _prep_in_maps(cls_mu_queue, cls_sc_queue, inp_mu, inp_sc, cls_idx):
    perm = np.asarray(cls_idx, dtype=np.int64)
    mu_g = np.asarray(cls_mu_queue, dtype=np.float32)[perm]
    sc_g = np.asarray(cls_sc_queue, dtype=np.float32)[perm]
    isc_g = np.asarray(inp_sc, dtype=np.float32).T[perm]  # [200, 512]
    impu = np.ascontiguousarray(np.asarray(inp_mu, dtype=np.float32))

    in_maps = []
    for k in range(N_CORES):
        cs = slice(k * CPC, (k + 1) * CPC)
        in_maps.append(
            {
                "qmu": np.ascontiguousarray(mu_g[cs]).reshape(INP_BASE, D),
                "impu": impu,
                "qsc": np.ascontiguousarray(sc_g[cs]),
                "isc": np.ascontiguousarray(isc_g[cs]),
            }
        )
    return in_maps, perm


def kernel_with_info(inputs: dict, trace: bool = False):
    from concourse import bass_utils

    nc = get_nc()
    in_maps, perm = _prep_in_maps(**inputs)
    res = bass_utils.run_bass_kernel_spmd(
        nc,
        in_maps,
        core_ids=list(range(N_CORES)),
        trace=trace,
    )

    cls_mu_queue = np.asarray(inputs["cls_mu_queue"], dtype=np.float32)
    cls_sc_queue = np.asarray(inputs["cls_sc_queue"], dtype=np.float32)
    out = np.empty((N_CLASS, N_MU, D + 1), dtype=np.float32)
    out[:, :, :D] = cls_mu_queue
    out[:, :, D] = cls_sc_queue
    for k in range(N_CORES):
        cls = perm[k * CPC : (k + 1) * CPC]
        out[cls, :, :D] = res.results[k]["out_mu"]
        out[cls, :, D] = res.results[k]["out_sc"]
    return out, res


def kernel(**inputs) -> np.ndarray:
    out, _ = kernel_with_info(inputs, trace=False)
    return out


# revision 9
# speedup vs baseline: 1.1686x; 1.1686x over previous
"""Trainium2 Bass kernel for nn_Memory_27882927686265 (scatter_memory).

Per-class sort-merge queue update:
  concat 1024 queue scores + 512 input scores, stable-descending top-1024,
  gather the corresponding 512-wide mu rows, scatter back per class.

Sharding: 200 classes split 25-per-core across 8 NeuronCores; inp_mu
replicated per core.

Device algorithm per core (classes on partitions 0..24):
  1. Copy queue-mu rows + inp_mu into one Internal DRAM slab (indirect DMA
     under this runtime only resolves dynamic offsets against Internal
     tensors, not ExternalInputs), giving a single gather index space.
  2. DVE iterative top-8 (max / max_index / match_replace), 128 rounds ->
     stable descending sort of all 1536 scores per class (ties resolved by
     ascending index, matching jax.lax.top_k).
  3. Per 128-rank block: DVE 32x32 block-transpose of the index block to a
     partition-major [128, 25] layout, map local indices to slab rows, then
     per class one indirect DMA gathers 128 rows (2KB each) into SBUF and a
     contiguous DMA stores them to the output.
"""

import threading

import numpy as np

N_CLASS = 200
N_MU = 1024
D = 512
K = 512
N_CORES = 8
CPC = N_CLASS // N_CORES  # classes per core = 25
NTOT = N_MU + K  # 1536
N_SRC_ROWS = CPC * N_MU + K  # 26112
INP_BASE = CPC * N_MU  # 25600
N_BLOCKS = N_MU // 128  # 8

_lock = threading.Lock()
_cache = {}


def _build_nc():
    import concourse.bacc as bacc
    import concourse.mybir as mybir
    import concourse.tile as tile
    from concourse import bass

    nc = bacc.Bacc(
        "TRN2",
        target_bir_lowering=False,
        debug=False,
        num_devices=N_CORES,
    )

    qmu = nc.dram_tensor("qmu", [INP_BASE, D], mybir.dt.float32, kind="ExternalInput")
    impu = nc.dram_tensor("impu", [K, D], mybir.dt.float32, kind="ExternalInput")
    qsc = nc.dram_tensor("qsc", [CPC, N_MU], mybir.dt.float32, kind="ExternalInput")
    isc = nc.dram_tensor("isc", [CPC, K], mybir.dt.float32, kind="ExternalInput")
    out_mu = nc.dram_tensor(
        "out_mu", [CPC, N_MU, D], mybir.dt.float32, kind="ExternalOutput"
    )
    out_sc = nc.dram_tensor(
        "out_sc", [CPC, N_MU], mybir.dt.float32, kind="ExternalOutput"
    )
    # Internal slab: [queue rows of all 25 classes | inp_mu rows].
    islab = nc.dram_tensor("islab", [N_SRC_ROWS, D], mybir.dt.float32)

    with tile.TileContext(nc) as tc, tc.tile_pool(name="persist", bufs=1) as pp:
        # Persistent tiles.
        s_tile = pp.tile([CPC, NTOT], mybir.dt.float32, name="scores", tag="scores")
        sc_sorted = pp.tile(
            [CPC, N_MU], mybir.dt.float32, name="sc_sorted", tag="sc_sorted"
        )
        # Per-block index tiles: [32, 128] so the DVE 32x32 block transpose
        # applies directly; only rows :25 carry data.
        idx_blk = [
            pp.tile([32, 128], mybir.dt.uint32, name=f"idx_blk{b}", tag=f"idx_blk{b}")
            for b in range(N_BLOCKS)
        ]
        idx_blk_t = [
            pp.tile(
                [32, 128], mybir.dt.uint32, name=f"idx_blk_t{b}", tag=f"idx_blk_t{b}"
            )
            for b in range(N_BLOCKS)
        ]
        # Per-column class base (1024*c), as f32 for the DVE float ALU.
        base_cls = pp.tile([128, CPC], mybir.dt.float32, name="base_cls", tag="base")

        nc.gpsimd.iota(
            base_cls[:],
            pattern=[[N_MU, CPC]],
            base=0,
            channel_multiplier=0,
            allow_small_or_imprecise_dtypes=True,
        )
        for b in range(N_BLOCKS):
            nc.gpsimd.memset(idx_blk[b][:], 0)

        # Load scores: [q | inp] per class.
        nc.sync.dma_start(s_tile[:, :N_MU], qsc.ap())
        nc.sync.dma_start(s_tile[:, N_MU:], isc.ap())

        # Fill the slab (DRAM->DRAM) behind the score loads, from the scalar
        # engine's DMA ring so it doesn't head-block the sync ring. Split
        # per-block so the first gathers don't wait for the whole 53MB.
        slab_step = INP_BASE // N_BLOCKS
        for b in range(N_BLOCKS):
            nc.scalar.dma_start(
                islab.ap()[b * slab_step : (b + 1) * slab_step, :],
                qmu.ap()[b * slab_step : (b + 1) * slab_step, :],
            )
        nc.scalar.dma_start(islab.ap()[INP_BASE:, :], impu.ap())

        # Stable descending selection, 8 at a time.
        for t in range(N_MU // 8):
            b, w = divmod(t, 16)
            mx = sc_sorted[:CPC, 8 * t : 8 * t + 8]
            nc.vector.max(out=mx, in_=s_tile[:CPC, :])
            nc.vector.max_index(
                out=idx_blk[b][:CPC, 8 * w : 8 * w + 8],
                in_max=mx,
                in_values=s_tile[:CPC, :],
            )
            if t != N_MU // 8 - 1:
                nc.vector.match_replace(
                    out=s_tile[:CPC, :],
                    in_to_replace=mx,
                    in_values=s_tile[:CPC, :],
                    imm_value=-1.0,
                )

        with (
            tc.tile_pool(name="stage", bufs=8) as stage_pool,
            tc.tile_pool(name="idxg", bufs=2) as idx_pool,
        ):
            for b in range(N_BLOCKS):
                # Transpose [25,128] block (padded to 32 rows) to partition-major.
                nc.vector.transpose(out=idx_blk_t[b][:], in_=idx_blk[b][:])
                tpos = idx_pool.tile([128, CPC], mybir.dt.float32, tag="tpos")
                for g in range(4):
                    nc.vector.tensor_copy(
                        out=tpos[32 * g : 32 * g + 32, :],
                        in_=idx_blk_t[b][:, 32 * g : 32 * g + CPC],
                    )
                # Slab row: idx < 1024 -> 1024*c + idx ; else idx - 1024 + 25600
                mask = idx_pool.tile([128, CPC], mybir.dt.uint32, tag="mask")
                addq = idx_pool.tile([128, CPC], mybir.dt.float32, tag="addq")
                gidxf = idx_pool.tile([128, CPC], mybir.dt.float32, tag="gidxf")
                gidx = idx_pool.tile([128, CPC], mybir.dt.int32, tag="gidx")
                nc.vector.tensor_scalar(
                    mask[:], tpos[:], float(N_MU), None, op0=mybir.AluOpType.is_lt
                )
                nc.vector.tensor_tensor(
                    out=addq[:], in0=tpos[:], in1=base_cls[:], op=mybir.AluOpType.add
                )
                nc.vector.tensor_scalar(
                    gidxf[:],
                    tpos[:],
                    float(INP_BASE - N_MU),
                    None,
                    op0=mybir.AluOpType.add,
                )
                nc.vector.copy_predicated(gidxf[:], mask[:], addq[:])
                nc.vector.tensor_copy(out=gidx[:], in_=gidxf[:])

                for c in range(CPC):
                    stage = stage_pool.tile([128, D], mybir.dt.float32, tag="stage")
                    nc.gpsimd.indirect_dma_start(
                        out=stage[:],
                        out_offset=None,
                        in_=islab.ap(),
                        in_offset=bass.IndirectOffsetOnAxis(
                            ap=gidx[:, c : c + 1], axis=0
                        ),
                    )
                    nc.sync.dma_start(
                        out_mu.ap()[c, 128 * b : 128 * (b + 1), :], stage[:]
                    )

        nc.sync.dma_start(out_sc.ap(), sc_sorted[:CPC, :])

    nc.compile()
    return nc


def get_nc():
    with _lock:
        if "nc" not in _cache:
            _cache["nc"] = _build_nc()
        return _cache["nc"]


def _prep_in_maps(cls_mu_queue, cls_sc_queue, inp_mu, inp_sc, cls_idx):
    perm = np.asarray(cls_idx, dtype=np.int64)
    mu_g = np.asarray(cls_mu_queue, dtype=np.float32)[perm]
    sc_g = np.asarray(cls_sc_queue, dtype=np.float32)[perm]
    isc_g = np.asarray(inp_sc, dtype=np.float32).T[perm]  # [200, 512]
    impu = np.ascontiguousarray(np.asarray(inp_mu, dtype=np.float32))

    in_maps = []
    for k in range(N_CORES):
        cs = slice(k * CPC, (k + 1) * CPC)
        in_maps.append(
            {
                "qmu": np.ascontiguousarray(mu_g[cs]).reshape(INP_BASE, D),
                "impu": impu,
                "qsc": np.ascontiguousarray(sc_g[cs]),
                "isc": np.ascontiguousarray(isc_g[cs]),
            }
        )
    return in_maps, perm


def kernel_with_info(inputs: dict, trace: bool = False):
    from concourse import bass_utils

    nc = get_nc()
    in_maps, perm = _prep_in_maps(**inputs)
    res = bass_utils.run_bass_kernel_spmd(
        nc,
        in_maps,
        core_ids=list(range(N_CORES)),
        trace=trace,
    )

    cls_mu_queue = np.asarray(inputs["cls_mu_queue"], dtype=np.float32)
    cls_sc_queue = np.asarray(inputs["cls_sc_queue"], dtype=np.float32)
    out = np.empty((N_CLASS, N_MU, D + 1), dtype=np.float32)
    out[:, :, :D] = cls_mu_queue
    out[:, :, D] = cls_sc_queue
    for k in range(N_CORES):
        cls = perm[k * CPC : (k + 1) * CPC]
        out[cls, :, :D] = res.results[k]["out_mu"]
        out[cls, :, D] = res.results[k]["out_sc"]
    return out, res


def kernel(**inputs) -> np.ndarray:
    out, _ = kernel_with_info(inputs, trace=False)
    return out
